# revision 31
# baseline (speedup 1.0000x reference)
"""GAT layer (4 heads) on 8 Trainium2 NeuronCores via Bass/Tile — v4.

Strategy (per sharding hint): destination nodes sharded across 8 cores; edges
partitioned by destination so segment-softmax / segment-sum are core-local.

v2 key idea: h = x@W has rank <= IN_DIM, so the per-edge gather ships the
128-wide x row instead of the 256-wide h row, and W is applied AFTER
aggregation:  agg_x[d,h,:] = sum_e alpha[e,h]*x[src_e,:] -> agg_h = agg_x@W_h

v4 key ideas on top of v2 (141us -> ~100us single-shot):
  * The host BIN-PACKS dsts into 8-dst blocks (snake-stratified by degree +
    repair swaps) so every block's edge count is <= 128 and nearly full: one
    128-edge tile per block, no tile quantization padding (tcols 977 -> 848)
    => -13% HBM traffic and PE work.
  * BLK=8 shrinks the one-hot window: DVE builds S2 (16-wide dup-pairs) +
    Salpha (32-wide) per tile = 4x less DVE work than BLK=32.
  * 16 blocks per epilogue group, quad-stacked 4-per-128-partitions in 4
    psum col-regions of [128,(q,d,h)] x [x|1]; identical epilogue shapes to
    v2 (nrm/transpose/W/relu/fc on 128-dst groups).
  * Normalize = one fused DVE mult (rc free-varying) instead of 4 ACT ops.
  * Epilogue split into norm (depth 1) and tail a/b/c stages interleaved
    with the next groups' agg matmuls; one psum bank per tail (transposed
    blocks bf16 overwritten in place by W outputs after the xpT evac).
  * Output stores batched 4 groups per DMA and issued on the ACT hwdge
    queue so g-row loads stream back-to-back on the SP queue; g-rows
    loaded 2 groups per DMA, 3 pairs ahead.
  * Output in bf16 (halves store traffic; rel-err budget is 2e-2).

Device programs:
  P1 (node-sharded): attention logits only. va_s = per-head <W_h, att_src_h>
     column vectors are built on device, then as/ad = x @ [va_s|va_d];
     also emits x in bf16. Output per node: xb[128] + as[4] + ad[4].
  -- host performs the per-edge gather as a byte-level index shuffle of the
     device-computed table (device-side gather DMA is non-functional in this
     environment: dma_gather and all Pool/GpSimd instructions need Q7 ucode),
     building rows [x |1| as | ad] per edge slot.
  P2 (dst-sharded): see v4 notes above.

All numerics are computed on device; the host only packs/pads/indexes.
"""

import dataclasses
import sys

sys.path.insert(0, "/opt/trn_rl_repo")

import ml_dtypes
import numpy as np

import concourse.bass as bass
import concourse.mybir as mybir
from concourse import tile
from concourse.bass_utils import run_bass_kernel_spmd

# problem shape (hardcoded per spec)
N, E = 50000, 800000
IN_DIM, HID, HEADS, OUT_DIM = 128, 64, 4, 128
NEG_SLOPE = 0.2
NCORES = 8
DPC = N // NCORES          # dst nodes per core: 6250
BLK = 8                    # dst block width (one-hot window); block == tile
GBLK = 16                  # blocks per epilogue group (4 psum regions x 4 quads)
NG = 53                    # groups per core
NBLK = NG * GBLK           # 848 blocks = tile columns per core
DHW = HEADS * BLK          # 32: (d,h)-interleaved one-hot width per block
ROW = IN_DIM + 1 + 2 * HEADS + 1   # 138: x(128) | 1 | as(4) | ad(4) | pad
ONE_OFF = IN_DIM           # 128
AS_OFF = IN_DIM + 1        # 129
AD_OFF = AS_OFF + HEADS    # 133
HH = HEADS * HID           # 256
BF16 = ml_dtypes.bfloat16
AS_PAD = -100.0            # poison 'as' => alpha ~ exp(-20) ~ 2e-9


def _split_waits(nc, max_waits=1):
    """This container's walrus only encodes one sync-wait per instruction;
    hoist excess waits onto NoOps inserted before the instruction."""
    n = 0
    for f in nc.m.functions:
        for blk in f.blocks:
            insts = blk.instructions
            idx = 0
            while idx < len(insts):
                inst = insts[idx]
                si = inst.sync_info
                waits = list(si.on_wait) if (si is not None and si.on_wait) else []
                if len(waits) > max_waits:
                    keep = waits[-max_waits:]
                    extra = waits[:-max_waits]
                    pos = idx
                    for j in range(0, len(extra), max_waits):
                        nop = mybir.InstNoOp(
                            name=f"waitsplit{n}_{inst.name}",
                            sync_info=mybir.SyncInfo(
                                on_wait=extra[j : j + max_waits], on_update=[]
                            ),
                            engine=inst.engine,
                            bass_nofuse=True,
                        )
                        nc.register_instruction(nop, overwrite=True)
                        insts.insert(pos, nop)
                        pos += 1
                        n += 1
                    inst.sync_info = mybir.SyncInfo(
                        on_wait=keep, on_update=list(si.on_update or [])
                    )
                    idx = pos + 1
                else:
                    idx += 1
    return n


def _v(ap, dims, offset=0):
    """Free-dim view of an AP: dims = [(step, count), ...] in elements,
    innermost last; partition dim kept."""
    return dataclasses.replace(
        ap, offset=ap.offset + offset, ap=[ap.ap[0]] + [[s, c] for s, c in dims]
    )


def build_prog1():
    """Per-core: [xb | as | ad] table rows for its 6250 nodes."""
    nc = bass.Bass()
    f32, bf16 = mybir.dt.float32, mybir.dt.bfloat16
    xTs = nc.dram_tensor("xTs", [IN_DIM, DPC], f32, kind="ExternalInput")
    xs = nc.dram_tensor("xs", [DPC, IN_DIM], f32, kind="ExternalInput")
    wnat = nc.dram_tensor("wnat", [IN_DIM, HH], f32, kind="ExternalInput")
    attsr = nc.dram_tensor("attsr", [128, HH], f32, kind="ExternalInput")
    attdr = nc.dram_tensor("attdr", [128, HH], f32, kind="ExternalInput")
    xb_out = nc.dram_tensor("xb", [DPC, IN_DIM], bf16, kind="ExternalOutput")
    asad = nc.dram_tensor("asad", [DPC, 2 * HEADS], bf16, kind="ExternalOutput")

    ntile = [128] * (DPC // 128) + ([DPC % 128] if DPC % 128 else [])
    with tile.TileContext(nc) as tc:
        with (
            tc.tile_pool(name="cst", bufs=1) as cst,
            tc.tile_pool(name="sb", bufs=3) as sb,
            tc.tile_pool(name="ps", bufs=3, space="PSUM") as ps,
        ):
            xT_sb = cst.tile([IN_DIM, DPC], f32)
            nc.sync.dma_start(out=xT_sb[:], in_=xTs[:, :])
            w_sb = cst.tile([IN_DIM, HH], f32)
            nc.sync.dma_start(out=w_sb[:], in_=wnat[:, :])
            as_sb = cst.tile([128, HH], f32)
            nc.sync.dma_start(out=as_sb[:], in_=attsr[:, :])
            ad_sb = cst.tile([128, HH], f32)
            nc.sync.dma_start(out=ad_sb[:], in_=attdr[:, :])

            # va[f, h] = sum_c W[f, h*64+c] * att[h, c]  (both heads packs)
            va_sb = cst.tile([IN_DIM, 2 * HEADS], f32)
            for rep_sb, coff in ((as_sb, 0), (ad_sb, HEADS)):
                t_sb = sb.tile([IN_DIM, HH], f32, name="t_sb", tag="tmul")
                nc.vector.tensor_mul(t_sb[:], w_sb[:], rep_sb[:])
                nc.vector.reduce_sum(
                    out=va_sb[:, coff : coff + HEADS],
                    in_=_v(t_sb[:], [(HID, HEADS), (1, HID)]),
                    axis=mybir.AxisListType.X,
                )
            vab_sb = cst.tile([IN_DIM, 2 * HEADS], bf16)
            nc.scalar.copy(out=vab_sb[:], in_=va_sb[:])
            # bf16 xT for the as/ad matmuls
            xTb_sb = cst.tile([IN_DIM, DPC], bf16)
            nc.scalar.copy(out=xTb_sb[:], in_=xT_sb[:])

            # batched x-row load / bf16 convert / store (tiled [128, t*128])
            NT, TAIL = DPC // 128, DPC % 128   # 48 full tiles + 106 rows
            xr_all = cst.tile([128, (DPC // 128 + 1) * IN_DIM], f32)
            nc.sync.dma_start(
                out=_v(xr_all[:], [(IN_DIM, NT), (1, IN_DIM)]),
                in_=_v(xs[0:128, :], [(128 * IN_DIM, NT), (1, IN_DIM)]),
            )
            nc.sync.dma_start(
                out=xr_all[:TAIL, NT * IN_DIM :],
                in_=xs[NT * 128 : NT * 128 + TAIL, :],
            )
            xb_all = cst.tile([128, (DPC // 128 + 1) * IN_DIM], bf16)
            nc.vector.tensor_copy(out=xb_all[:], in_=xr_all[:])
            nc.sync.dma_start(
                out=_v(xb_out[0:128, :], [(128 * IN_DIM, NT), (1, IN_DIM)]),
                in_=_v(xb_all[:], [(IN_DIM, NT), (1, IN_DIM)]),
            )
            nc.sync.dma_start(
                out=xb_out[NT * 128 : NT * 128 + TAIL, :],
                in_=xb_all[:TAIL, NT * IN_DIM :],
            )

            ab_all = cst.tile([128, len(ntile) * 2 * HEADS], bf16)
            n0 = 0
            for ti, nt in enumerate(ntile):
                a_ps = ps.tile([128, 2 * HEADS], f32, name="a_ps")
                nc.tensor.matmul(
                    out=a_ps[:nt, :],
                    lhsT=xTb_sb[:, n0 : n0 + nt],
                    rhs=vab_sb[:],
                    start=True,
                    stop=True,
                )
                nc.scalar.copy(
                    out=ab_all[:nt, ti * 2 * HEADS : (ti + 1) * 2 * HEADS],
                    in_=a_ps[:nt, :],
                )
                n0 += nt
            nc.sync.dma_start(
                out=_v(asad[0:128, :], [(128 * 2 * HEADS, NT), (1, 2 * HEADS)]),
                in_=_v(ab_all[:], [(2 * HEADS, NT), (1, 2 * HEADS)]),
            )
            nc.sync.dma_start(
                out=asad[NT * 128 : NT * 128 + TAIL, :],
                in_=ab_all[:TAIL, NT * 2 * HEADS : (NT + 1) * 2 * HEADS],
            )
    _split_waits(nc)
    return nc


def build_prog2(reps=1):
    """Per-core edge program v4: one 128-edge tile per 8-dst block, 16 blocks
    per epilogue group quad-stacked into 4 psum col-regions of 128 (d,h)
    partitions each. Host bin-packs dsts into blocks so tiles are ~full.
    """
    nc = bass.Bass()
    f32, bf16 = mybir.dt.float32, mybir.dt.bfloat16
    tcols = NBLK
    g_dram = nc.dram_tensor("g", [128, tcols * ROW], bf16, kind="ExternalInput")
    dm2_dram = nc.dram_tensor("dm2", [128, tcols * 2], bf16, kind="ExternalInput")
    iota2_dram = nc.dram_tensor("iota2", [128, 2 * BLK], bf16, kind="ExternalInput")
    w_dram = nc.dram_tensor("wnat", [IN_DIM, HH], f32, kind="ExternalInput")
    fw1_dram = nc.dram_tensor("fw1", [128, OUT_DIM], f32, kind="ExternalInput")
    fw2_dram = nc.dram_tensor("fw2", [128, OUT_DIM], f32, kind="ExternalInput")
    fb_dram = nc.dram_tensor("fbr", [128, OUT_DIM], f32, kind="ExternalInput")
    id_dram = nc.dram_tensor("id128", [128, 128], bf16, kind="ExternalInput")
    out_dram = nc.dram_tensor("out", [NG * 128, OUT_DIM], bf16, kind="ExternalOutput")

    with tile.TileContext(nc) as tc:
        with (
            tc.tile_pool(name="cst", bufs=1) as cst,
            tc.tile_pool(name="sb", bufs=10) as sb,
            tc.tile_pool(name="sb2", bufs=8) as sb2,
            tc.tile_pool(name="ps", bufs=3, space="PSUM") as ps,
            tc.tile_pool(name="ps2", bufs=3, space="PSUM") as ps2,
        ):
            dm2_sb = cst.tile([128, tcols * 2], bf16)
            nc.sync.dma_start(out=dm2_sb[:], in_=dm2_dram[:, :])
            iota2_sb = cst.tile([128, 2 * BLK], bf16)
            nc.sync.dma_start(out=iota2_sb[:], in_=iota2_dram[:, :])
            w_sb = cst.tile([IN_DIM, HH], f32)
            nc.sync.dma_start(out=w_sb[:], in_=w_dram[:, :])
            wb_sb = cst.tile([IN_DIM, HH], bf16)
            nc.scalar.copy(out=wb_sb[:], in_=w_sb[:])
            fw1_sb = cst.tile([128, OUT_DIM], f32)
            nc.sync.dma_start(out=fw1_sb[:], in_=fw1_dram[:, :])
            fw2_sb = cst.tile([128, OUT_DIM], f32)
            nc.sync.dma_start(out=fw2_sb[:], in_=fw2_dram[:, :])
            fw1b_sb = cst.tile([128, OUT_DIM], bf16)
            nc.scalar.copy(out=fw1b_sb[:], in_=fw1_sb[:])
            fw2b_sb = cst.tile([128, OUT_DIM], bf16)
            nc.scalar.copy(out=fw2b_sb[:], in_=fw2_sb[:])
            fb_sb = cst.tile([128, OUT_DIM], f32)
            nc.sync.dma_start(out=fb_sb[:], in_=fb_dram[:, :])
            id_sb = cst.tile([128, 128], bf16)
            nc.sync.dma_start(out=id_sb[:], in_=id_dram[:, :])
            ones1_sb = cst.tile([128, 128], bf16)
            nc.vector.memset(ones1_sb[:], 1.0)
            fbb_sb = cst.tile([128, OUT_DIM], bf16)
            nc.scalar.copy(out=fbb_sb[:], in_=fb_sb[:])

            def load_group(pi):
                    # one DMA covers two 16-block groups (pair pi)
                    soff = pi * 2 * GBLK
                    ncols = min(2 * GBLK, NBLK - soff)
                    g_sb = sb.tile([128, 2 * GBLK * ROW], bf16, name="g_sb", tag="g")
                    nc.sync.dma_start(
                        out=g_sb[:, : ncols * ROW],
                        in_=g_dram[:, soff * ROW : (soff + ncols) * ROW],
                    )
                    return g_sb

            def prep(gi, g_pair):
                    # alpha/one-hot prep for one 16-block group
                    soff = gi * GBLK
                    sT = GBLK
                    g_sb = g_pair[:, (gi % 2) * GBLK * ROW : ((gi % 2) + 1) * GBLK * ROW]
                    # alpha = exp(leakyrelu(as + ad)) : [e, (t,h)]
                    q_sb = sb.tile([128, sT * HEADS], bf16, name="q_sb", tag="q")
                    nc.vector.tensor_tensor(
                        out=q_sb[:],
                        in0=_v(g_sb, [(ROW, sT), (1, HEADS)], offset=AS_OFF),
                        in1=_v(g_sb, [(ROW, sT), (1, HEADS)], offset=AD_OFF),
                        op=mybir.AluOpType.add,
                    )
                    lr_sb = sb.tile([128, sT * HEADS], bf16, name="lr_sb", tag="lr")
                    nc.vector.scalar_tensor_tensor(
                        out=lr_sb[:], in0=q_sb[:], scalar=NEG_SLOPE, in1=q_sb[:],
                        op0=mybir.AluOpType.mult, op1=mybir.AluOpType.max,
                    )
                    al_sb = sb.tile([128, sT * HEADS], bf16, name="al_sb", tag="al")
                    nc.scalar.activation(
                        out=al_sb[:], in_=lr_sb[:],
                        func=mybir.ActivationFunctionType.Exp,
                    )
                    # S2[e, (t,d,j2)] = (dm[e,t] == d), j duplicated pair
                    s2_sb = sb.tile([128, sT * 2 * BLK], bf16, name="s2_sb", tag="s2")
                    nc.vector.tensor_tensor(
                        out=s2_sb[:],
                        in0=_v(iota2_sb[:], [(0, sT), (1, 2 * BLK)]),
                        in1=_v(dm2_sb[:], [(2, sT), (0, BLK), (1, 2)], offset=soff * 2),
                        op=mybir.AluOpType.is_equal,
                    )
                    # Salpha[e, (t, d, h4)] = S2 * alpha, two packed head-pair muls
                    sa_sb = sb.tile([128, sT * DHW], bf16, name="sa_sb", tag="sa")
                    for p2 in (0, 2):
                        nc.vector.tensor_tensor(
                            out=_v(sa_sb[:], [(DHW, sT), (HEADS, BLK), (1, 2)],
                                   offset=p2),
                            in0=_v(s2_sb[:], [(2 * BLK, sT), (2, BLK), (1, 2)]),
                            in1=_v(al_sb[:], [(HEADS, sT), (0, BLK), (1, 2)],
                                   offset=p2),
                            op=mybir.AluOpType.mult,
                        )
                    return g_sb, sa_sb

            def agg_group(ctx, b0, b1, agg_ps=None):
                    # block b (0..15) -> psum col-region b//4 (256-col pitch),
                    # partition quad (b%4)*32. col 128 of each region = den.
                    g_sb, sa_sb = ctx
                    if agg_ps is None:
                        agg_ps = ps.tile([128, 4 * 256], f32, name="agg_ps", tag="agg")
                    for b in range(b0, b1):
                        qo = (b % 4) * DHW
                        co = (b // 4) * 256
                        nc.tensor.matmul(
                            out=agg_ps[qo : qo + DHW, co : co + IN_DIM + 1],
                            lhsT=sa_sb[:, b * DHW : (b + 1) * DHW],
                            rhs=g_sb[:, b * ROW : b * ROW + IN_DIM + 1],
                            start=True,
                            stop=True,
                            tile_position=(0, qo),
                        )
                    return agg_ps

            def norm_stage(gi, agg_ps):
                    import os
                    if os.environ.get('SKIP_NORM'):
                        return None
                    # reciprocal over the 4 denominator cols + one fused
                    # normalize over all 4 regions (rc free-varying on DVE);
                    # frees agg_ps afterwards.
                    rc_sb = sb2.tile([128, 4], f32, name="rc", tag="rc")
                    nc.vector.reciprocal(
                        rc_sb[:],
                        _v(agg_ps[:], [(256, 4), (1, 1)], offset=IN_DIM),
                    )
                    nrm_sb = sb2.tile([128, 4 * 128], bf16, name="nrm", tag="nrm")
                    nc.vector.tensor_tensor(
                        out=nrm_sb[:],
                        in0=_v(agg_ps[:], [(256, 4), (1, IN_DIM)]),
                        in1=_v(rc_sb[:], [(1, 4), (0, IN_DIM)]),
                        op=mybir.AluOpType.mult,
                    )
                    return nrm_sb

            def tail_a(gi, nrm_sb):
                    if nrm_sb is None:
                        return None, None
                    # transposes + xpT evac. one psum bank per group: bytes
                    # 0:1024 first hold the 4 transposed blocks (bf16), then
                    # the per-head W outputs (fp32) overwrite them after the
                    # xpT evac; fc accumulates into bytes 1024:1536.
                    abo_ps = ps2.tile([128, 512], f32, name="abo_ps", tag="abo")
                    xp_ps = abo_ps[:].bitcast(bf16)   # [128, 1024] bf16 view
                    for r in range(4):
                        nc.tensor.transpose(
                            out=xp_ps[:, r * 128 : (r + 1) * 128],
                            in_=nrm_sb[:, r * 128 : (r + 1) * 128],
                            identity=id_sb[:],
                        )
                    xpT_sb = sb2.tile([128, 4 * 128], bf16, name="xpT", tag="xpT")
                    nc.scalar.copy(out=xpT_sb[:], in_=xp_ps[:, 0:512])
                    return abo_ps, xpT_sb

            def tail_b(gi, abo_ps, xpT_sb):
                    if abo_ps is None:
                        return None, None
                    # per-head W matmuls: xpT col = r*128 + q*32 + d*4 + h
                    for h in range(HEADS):
                        nc.tensor.matmul(
                            out=abo_ps[(h % 2) * HID : (h % 2 + 1) * HID,
                                       (h // 2) * 128 : (h // 2 + 1) * 128],
                            lhsT=wb_sb[:, h * HID : (h + 1) * HID],
                            rhs=_v(xpT_sb[:], [(128, 4), (DHW, 4), (HEADS, BLK)],
                                   offset=h),
                            start=True,
                            stop=True,
                        )
                    rl_sb = sb2.tile([128, 2 * 128], bf16, name="rl", tag="rl")
                    nc.scalar.activation(
                        out=rl_sb[:], in_=abo_ps[:, 0:256],
                        func=mybir.ActivationFunctionType.Relu,
                    )
                    return abo_ps, rl_sb

            ou_state = {}

            def tail_c(gi, abo_ps, rl_sb):
                    if abo_ps is None:
                        return
                    o_ps = abo_ps[:, 256:384]
                    nc.tensor.matmul(
                        out=o_ps, lhsT=rl_sb[:, 0:128], rhs=fw1b_sb[:],
                        start=True, stop=False,
                    )
                    nc.tensor.matmul(
                        out=o_ps, lhsT=rl_sb[:, 128:256], rhs=fw2b_sb[:],
                        start=False, stop=False,
                    )
                    # + fc_b via a 1-partition ones matmul (PSUM accumulate)
                    nc.tensor.matmul(
                        out=o_ps, lhsT=ones1_sb[0:1, :], rhs=fbb_sb[0:1, :],
                        start=False, stop=True,
                    )
                    # batch 4 groups per output store so g-load dispatches
                    # rarely queue behind a store on the SP queue
                    if "tile" not in ou_state:
                        ou_state["tile"] = sb2.tile(
                            [128, 4 * OUT_DIM], bf16, name="ou4", tag="ou4"
                        )
                        ou_state["g0"] = gi
                    ou4 = ou_state["tile"]
                    sl = gi - ou_state["g0"]
                    nc.scalar.copy(
                        out=ou4[:, sl * OUT_DIM : (sl + 1) * OUT_DIM],
                        in_=abo_ps[:, 256:384],
                    )
                    if sl == 3 or gi == NG - 1:
                        g0 = ou_state["g0"]
                        nc.scalar.dma_start(
                            out=_v(
                                out_dram[g0 * 128 : g0 * 128 + 128, :],
                                [(128 * OUT_DIM, sl + 1), (1, OUT_DIM)],
                            ),
                            in_=_v(ou4[:], [(OUT_DIM, sl + 1), (1, OUT_DIM)]),
                        )
                        ou_state.clear()

            # software pipeline, two epilogue stages: after emitting group g's
            # prep+agg, emit norm (DVE) for g-1 and the PE/ACT tail for g-2.
            # Keeps every engine's in-order stream free of head-of-line waits.
            rep_cm = tc.For_i(0, reps, 1) if reps > 1 else None
            if rep_cm is not None:
                rep_cm.__enter__()
            if True:
                NPAIR = (NG + 1) // 2
                PF = 3   # dma prefetch depth (pairs)
                loaded = {pi: load_group(pi) for pi in range(min(PF, NPAIR))}
                pend_norm = []   # (gi, agg_ps)
                pend_a = []      # (gi, nrm_sb)
                pend_b = []      # (gi, abo_ps, xpT_sb)
                pend_c = []      # (gi, abo_ps, rl_sb)
                def flush(n_keep_a=0, n_keep_b=0, n_keep_c=0):
                    while len(pend_a) > n_keep_a:
                        gp, nrm = pend_a.pop(0)
                        pend_b.append((gp,) + tail_a(gp, nrm))
                    while len(pend_b) > n_keep_b:
                        gp, abo, xpT = pend_b.pop(0)
                        pend_c.append((gp,) + tail_b(gp, abo, xpT))
                    while len(pend_c) > n_keep_c:
                        gp, abo, rl = pend_c.pop(0)
                        tail_c(gp, abo, rl)
                for gi in range(NG):
                    pi = gi // 2
                    if gi % 2 == 0 and pi + PF < NPAIR:
                        loaded[pi + PF] = load_group(pi + PF)
                    g_pair = loaded[pi] if gi % 2 == 0 else loaded.pop(pi)
                    ctx = prep(gi, g_pair)
                    # interleave tail PE work with this group's agg matmuls
                    if pend_a:
                        gp, nrm = pend_a.pop(0)
                        pend_b.append((gp,) + tail_a(gp, nrm))
                    agg = agg_group(ctx, 0, 8)
                    if pend_b:
                        gp, abo, xpT = pend_b.pop(0)
                        pend_c.append((gp,) + tail_b(gp, abo, xpT))
                    agg_group(ctx, 8, 16, agg)
                    if pend_c:
                        gp, abo, rl = pend_c.pop(0)
                        tail_c(gp, abo, rl)
                    pend_norm.append((gi, agg))
                    if len(pend_norm) > 1:
                        gp, aggp = pend_norm.pop(0)
                        pend_a.append((gp, norm_stage(gp, aggp)))
                for gp, aggp in pend_norm:
                    pend_a.append((gp, norm_stage(gp, aggp)))
                flush()
            if rep_cm is not None:
                rep_cm.__exit__(None, None, None)
    _split_waits(nc)
    return nc


def _pack_core(dsts_sorted_desc, deg, nbins, cap=128, max_items=BLK):
    """Snake-stratified assignment + repair swaps so every bin has
    <= max_items dsts and <= cap edges. Returns list of dst-lists."""
    bins = [[] for _ in range(nbins)]
    b, direction = 0, 1
    for d in dsts_sorted_desc:
        bins[b].append(d)
        b += direction
        if b == nbins:
            b, direction = nbins - 1, -1
        elif b < 0:
            b, direction = 0, 1
    sums = np.array([deg[bb].sum() for bb in bins], np.int64)
    cnt = np.array([len(bb) for bb in bins], np.int64)
    for _ in range(100000):
        over = np.where(sums > cap)[0]
        if len(over) == 0:
            break
        i = over[np.argmax(sums[over])]
        excess = sums[i] - cap
        done = False
        for a in sorted(bins[i], key=lambda d: -deg[d]):
            under = np.where(sums <= cap - 1)[0]
            ju = under[np.argsort(sums[under])]
            for j in ju[:64]:
                for bidx, bd in enumerate(bins[j]):
                    da, db = deg[a], deg[bd]
                    if da - db >= excess and sums[j] - db + da <= cap:
                        bins[i].remove(a)
                        bins[j].pop(bidx)
                        bins[i].append(bd)
                        bins[j].append(a)
                        sums[i] += db - da
                        sums[j] += da - db
                        done = True
                        break
                if done:
                    break
            if done:
                break
        if not done:
            for a in sorted(bins[i], key=lambda d: -deg[d]):
                room = np.where((sums + deg[a] <= cap) & (cnt < max_items))[0]
                if len(room):
                    j = room[np.argmax(sums[room])]
                    bins[i].remove(a)
                    bins[j].append(a)
                    sums[i] -= deg[a]
                    sums[j] += deg[a]
                    cnt[i] -= 1
                    cnt[j] += 1
                    done = True
                    break
            if not done:
                raise RuntimeError("bin packing failed; raise NBLK")
    assert (sums <= cap).all() and (cnt <= max_items).all()
    return bins


def _host_prep(edge_index):
    """Index-only prep v4: self loops, degree-balanced dst->core snake,
    per-core bin-packing of dsts into NBLK 8-dst/128-edge tiles.
    Returns per-core (sid, dmod) slot arrays plus row_dst[NCORES, NG*128]
    (destination node per output slot, -1 for empty)."""
    src = np.concatenate(
        [np.asarray(edge_index[0], np.int64), np.arange(N, dtype=np.int64)]
    ).astype(np.int32)
    dst = np.concatenate(
        [np.asarray(edge_index[1], np.int64), np.arange(N, dtype=np.int64)]
    ).astype(np.int32)
    deg = np.bincount(dst, minlength=N)
    # per-dst edge lists via dst sort
    order = np.argsort(dst, kind="stable")
    src_s = src[order]
    dst_start = np.zeros(N + 1, np.int64)
    dst_start[1:] = np.cumsum(deg)

    dorder = np.argsort(-deg, kind="stable")
    snake = np.tile(
        np.concatenate([np.arange(NCORES), np.arange(NCORES - 1, -1, -1)]),
        N // (2 * NCORES) + 1,
    )[:N]
    core_of = np.empty(N, np.int32)
    core_of[dorder] = snake

    planes = []
    row_dst = np.full((NCORES, NG * 128), -1, np.int64)
    for k in range(NCORES):
        dsts_k = dorder[core_of[dorder] == k]  # desc by degree
        bins = _pack_core(dsts_k, deg, NBLK)
        sid = np.full(NBLK * 128, N, np.int32)
        did = np.full(NBLK * 128, N, np.int32)
        dmod = np.zeros(NBLK * 128, np.int32)
        for b, bl in enumerate(bins):
            gi, bg = b // GBLK, b % GBLK
            o = b * 128
            pos = 0
            for di, d in enumerate(bl):
                c = deg[d]
                sid[o + pos : o + pos + c] = src_s[dst_start[d] : dst_start[d] + c]
                did[o + pos : o + pos + c] = d
                dmod[o + pos : o + pos + c] = di
                pos += c
                row_dst[k, gi * 128 + (bg // 4) * 32 + (bg % 4) * 8 + di] = d
            # poison slots point at empty dmod slots (or 0 if bin is full)
            if pos < 128:
                dmod[o + pos : o + 128] = len(bl) % BLK
        planes.append((sid, did, dmod))
    return planes, row_dst


def prepare_in2(x, edge_index, W, att_src, att_dst, bias, fc_w, fc_b):
    """Run prog1 + host index shuffle; returns (in2, tcols, blk_off, blk_T)."""
    x = np.asarray(x, np.float32)
    W = np.asarray(W, np.float32)
    att_src = np.asarray(att_src, np.float32)
    att_dst = np.asarray(att_dst, np.float32)
    bias = np.asarray(bias, np.float32)
    fc_w = np.asarray(fc_w, np.float32)
    fc_b = np.asarray(fc_b, np.float32)

    xT = np.ascontiguousarray(x.T)                             # [128, N]
    attsr = np.tile(att_src.reshape(1, -1), (128, 1)).astype(np.float32)
    attdr = np.tile(att_dst.reshape(1, -1), (128, 1)).astype(np.float32)

    # ---- program 1: per-node [xb | as | ad] table shards
    nc1 = build_prog1()
    in1 = []
    for k in range(NCORES):
        in1.append(
            {
                "xTs": np.ascontiguousarray(xT[:, k * DPC : (k + 1) * DPC]),
                "xs": np.ascontiguousarray(x[k * DPC : (k + 1) * DPC, :]),
                "wnat": W,
                "attsr": attsr,
                "attdr": attdr,
            }
        )
    r1 = run_bass_kernel_spmd(nc1, in1, core_ids=list(range(NCORES)))
    xb = np.empty((N + 1, IN_DIM), np.uint16)
    asad = np.empty((N + 1, 2 * HEADS), np.uint16)
    for k in range(NCORES):
        xb[k * DPC : (k + 1) * DPC] = r1.results[k]["xb"].view(np.uint16)
        asad[k * DPC : (k + 1) * DPC] = r1.results[k]["asad"].view(np.uint16)
    # poison row: x=0, as=AS_PAD, ad=0
    xb[N, :] = 0
    asad[N, :HEADS] = np.array(AS_PAD, BF16).view(np.uint16)
    asad[N, HEADS:] = 0

    # ---- host: per-edge plane assembly (byte-level index shuffle only)
    planes, row_dst = _host_prep(edge_index)
    tcols = NBLK
    one_bf16 = np.array(1.0, BF16).view(np.uint16)
    iota2 = np.tile(
        np.repeat(np.arange(BLK, dtype=np.float32), 2).astype(BF16), (128, 1)
    )
    id128 = np.eye(128, dtype=np.float32).astype(BF16)
    fbr = np.tile(fc_b.reshape(1, -1), (128, 1)).astype(np.float32)
    # note: bias input is all-zero in this problem; fold would go into the
    # relu stage if nonzero.
    assert np.all(bias == 0.0), "nonzero GAT bias not implemented in v2"
    in2 = []
    for k in range(NCORES):
        sid, did, dmod = planes[k]
        rows = np.empty((tcols * 128, ROW), np.uint16)
        rows[:, :IN_DIM] = xb[sid]
        rows[:, ONE_OFF] = one_bf16
        rows[:, AS_OFF : AS_OFF + HEADS] = asad[sid, :HEADS]
        rows[:, AD_OFF : AD_OFF + HEADS] = asad[did, HEADS:]
        rows[:, ROW - 1] = 0
        g = np.ascontiguousarray(
            rows.reshape(tcols, 128, ROW).transpose(1, 0, 2).reshape(128, tcols * ROW)
        )
        dm2 = np.ascontiguousarray(
            np.repeat(dmod.astype(np.float32).astype(BF16), 2)
            .reshape(tcols, 128, 2).transpose(1, 0, 2).reshape(128, tcols * 2)
        )
        in2.append(
            {
                "g": g.view(BF16),
                "dm2": dm2,
                "iota2": iota2,
                "wnat": W,
                "fw1": np.ascontiguousarray(fc_w[:128, :]),
                "fw2": np.ascontiguousarray(fc_w[128:, :]),
                "fbr": fbr,
                "id128": id128,
            }
        )

    return in2, row_dst


def run_gat(x, edge_index, W, att_src, att_dst, bias, fc_w, fc_b, reps=1):
    in2, row_dst = prepare_in2(
        x, edge_index, W, att_src, att_dst, bias, fc_w, fc_b
    )
    nc2 = build_prog2(reps=reps)
    r2 = run_bass_kernel_spmd(nc2, in2, core_ids=list(range(NCORES)))
    out = np.empty((N, OUT_DIM), np.float32)
    for k in range(NCORES):
        rows = np.asarray(r2.results[k]["out"], dtype=np.float32)  # [NG*128, OUT]
        valid = row_dst[k] >= 0
        out[row_dst[k][valid]] = rows[valid]
    return out


def kernel(x, edge_index, W, att_src, att_dst, bias, fc_w, fc_b):
    return run_gat(x, edge_index, W, att_src, att_dst, bias, fc_w, fc_b, reps=1)



# revision 37
# speedup vs baseline: 1.1112x; 1.1112x over previous
"""GAT layer (4 heads) on 8 Trainium2 NeuronCores via Bass/Tile — v4.

Strategy (per sharding hint): destination nodes sharded across 8 cores; edges
partitioned by destination so segment-softmax / segment-sum are core-local.

v2 key idea: h = x@W has rank <= IN_DIM, so the per-edge gather ships the
128-wide x row instead of the 256-wide h row, and W is applied AFTER
aggregation:  agg_x[d,h,:] = sum_e alpha[e,h]*x[src_e,:] -> agg_h = agg_x@W_h

v4 key ideas on top of v2 (141us -> ~100us single-shot):
  * The host BIN-PACKS dsts into 8-dst blocks (snake-stratified by degree +
    repair swaps) so every block's edge count is <= 128 and nearly full: one
    128-edge tile per block, no tile quantization padding (tcols 977 -> 848)
    => -13% HBM traffic and PE work.
  * BLK=8 shrinks the one-hot window: DVE builds S2 (16-wide dup-pairs) +
    Salpha (32-wide) per tile = 4x less DVE work than BLK=32.
  * 16 blocks per epilogue group, quad-stacked 4-per-128-partitions in 4
    psum col-regions of [128,(q,d,h)] x [x|1]; identical epilogue shapes to
    v2 (nrm/transpose/W/relu/fc on 128-dst groups).
  * Normalize = one fused DVE mult (rc free-varying) instead of 4 ACT ops.
  * Epilogue split into norm (depth 1) and tail a/b/c stages interleaved
    with the next groups' agg matmuls; one psum bank per tail (transposed
    blocks bf16 overwritten in place by W outputs after the xpT evac).
  * Output stores batched 4 groups per DMA and issued on the ACT hwdge
    queue so g-row loads stream back-to-back on the SP queue; g-row
    loads prefetched 5 groups ahead.
  * Output in bf16 (halves store traffic; rel-err budget is 2e-2).

Device programs:
  P1 (node-sharded): attention logits only. va_s = per-head <W_h, att_src_h>
     column vectors are built on device, then as/ad = x @ [va_s|va_d];
     also emits x in bf16. Output per node: xb[128] + as[4] + ad[4].
  -- host performs the per-edge gather as a byte-level index shuffle of the
     device-computed table (device-side gather DMA is non-functional in this
     environment: dma_gather and all Pool/GpSimd instructions need Q7 ucode),
     building rows [x |1| as | ad] per edge slot.
  P2 (dst-sharded): see v4 notes above.

All numerics are computed on device; the host only packs/pads/indexes.
"""

import dataclasses
import sys

sys.path.insert(0, "/opt/trn_rl_repo")

import ml_dtypes
import numpy as np

import concourse.bass as bass
import concourse.mybir as mybir
from concourse import tile
from concourse.bass_utils import run_bass_kernel_spmd

# problem shape (hardcoded per spec)
N, E = 50000, 800000
IN_DIM, HID, HEADS, OUT_DIM = 128, 64, 4, 128
NEG_SLOPE = 0.2
NCORES = 8
DPC = N // NCORES          # dst nodes per core: 6250
BLK = 8                    # dst block width (one-hot window); block == tile
GBLK = 16                  # blocks per epilogue group (4 psum regions x 4 quads)
NG = 53                    # groups per core
NBLK = NG * GBLK           # 848 blocks = tile columns per core
DHW = HEADS * BLK          # 32: (d,h)-interleaved one-hot width per block
ROW = IN_DIM + 1 + 2 * HEADS + 1   # 138: x(128) | 1 | as(4) | ad(4) | pad
ONE_OFF = IN_DIM           # 128
AS_OFF = IN_DIM + 1        # 129
AD_OFF = AS_OFF + HEADS    # 133
HH = HEADS * HID           # 256
BF16 = ml_dtypes.bfloat16
AS_PAD = -100.0            # poison 'as' => alpha ~ exp(-20) ~ 2e-9


def _split_waits(nc, max_waits=1):
    """This container's walrus only encodes one sync-wait per instruction;
    hoist excess waits onto NoOps inserted before the instruction."""
    n = 0
    for f in nc.m.functions:
        for blk in f.blocks:
            insts = blk.instructions
            idx = 0
            while idx < len(insts):
                inst = insts[idx]
                si = inst.sync_info
                waits = list(si.on_wait) if (si is not None and si.on_wait) else []
                if len(waits) > max_waits:
                    keep = waits[-max_waits:]
                    extra = waits[:-max_waits]
                    pos = idx
                    for j in range(0, len(extra), max_waits):
                        nop = mybir.InstNoOp(
                            name=f"waitsplit{n}_{inst.name}",
                            sync_info=mybir.SyncInfo(
                                on_wait=extra[j : j + max_waits], on_update=[]
                            ),
                            engine=inst.engine,
                            bass_nofuse=True,
                        )
                        nc.register_instruction(nop, overwrite=True)
                        insts.insert(pos, nop)
                        pos += 1
                        n += 1
                    inst.sync_info = mybir.SyncInfo(
                        on_wait=keep, on_update=list(si.on_update or [])
                    )
                    idx = pos + 1
                else:
                    idx += 1
    return n


def _v(ap, dims, offset=0):
    """Free-dim view of an AP: dims = [(step, count), ...] in elements,
    innermost last; partition dim kept."""
    return dataclasses.replace(
        ap, offset=ap.offset + offset, ap=[ap.ap[0]] + [[s, c] for s, c in dims]
    )


def build_prog1():
    """Per-core: [xb | as | ad] table rows for its 6250 nodes."""
    nc = bass.Bass()
    f32, bf16 = mybir.dt.float32, mybir.dt.bfloat16
    xTs = nc.dram_tensor("xTs", [IN_DIM, DPC], f32, kind="ExternalInput")
    xs = nc.dram_tensor("xs", [DPC, IN_DIM], f32, kind="ExternalInput")
    wnat = nc.dram_tensor("wnat", [IN_DIM, HH], f32, kind="ExternalInput")
    attsr = nc.dram_tensor("attsr", [128, HH], f32, kind="ExternalInput")
    attdr = nc.dram_tensor("attdr", [128, HH], f32, kind="ExternalInput")
    xb_out = nc.dram_tensor("xb", [DPC, IN_DIM], bf16, kind="ExternalOutput")
    asad = nc.dram_tensor("asad", [DPC, 2 * HEADS], bf16, kind="ExternalOutput")

    ntile = [128] * (DPC // 128) + ([DPC % 128] if DPC % 128 else [])
    with tile.TileContext(nc) as tc:
        with (
            tc.tile_pool(name="cst", bufs=1) as cst,
            tc.tile_pool(name="sb", bufs=3) as sb,
            tc.tile_pool(name="ps", bufs=2, space="PSUM") as ps,
        ):
            xT_sb = cst.tile([IN_DIM, DPC], f32)
            nc.sync.dma_start(out=xT_sb[:], in_=xTs[:, :])
            w_sb = cst.tile([IN_DIM, HH], f32)
            nc.sync.dma_start(out=w_sb[:], in_=wnat[:, :])
            as_sb = cst.tile([128, HH], f32)
            nc.sync.dma_start(out=as_sb[:], in_=attsr[:, :])
            ad_sb = cst.tile([128, HH], f32)
            nc.sync.dma_start(out=ad_sb[:], in_=attdr[:, :])

            # va[f, h] = sum_c W[f, h*64+c] * att[h, c]  (both heads packs)
            va_sb = cst.tile([IN_DIM, 2 * HEADS], f32)
            for rep_sb, coff in ((as_sb, 0), (ad_sb, HEADS)):
                t_sb = sb.tile([IN_DIM, HH], f32, name="t_sb", tag="tmul")
                nc.vector.tensor_mul(t_sb[:], w_sb[:], rep_sb[:])
                nc.vector.reduce_sum(
                    out=va_sb[:, coff : coff + HEADS],
                    in_=_v(t_sb[:], [(HID, HEADS), (1, HID)]),
                    axis=mybir.AxisListType.X,
                )
            vab_sb = cst.tile([IN_DIM, 2 * HEADS], bf16)
            nc.scalar.copy(out=vab_sb[:], in_=va_sb[:])
            # bf16 xT for the as/ad matmuls
            xTb_sb = cst.tile([IN_DIM, DPC], bf16)
            nc.scalar.copy(out=xTb_sb[:], in_=xT_sb[:])

            # batched x-row load / bf16 convert / store (tiled [128, t*128])
            NT, TAIL = DPC // 128, DPC % 128   # 48 full tiles + 106 rows
            xr_all = cst.tile([128, (DPC // 128 + 1) * IN_DIM], f32)
            nc.sync.dma_start(
                out=_v(xr_all[:], [(IN_DIM, NT), (1, IN_DIM)]),
                in_=_v(xs[0:128, :], [(128 * IN_DIM, NT), (1, IN_DIM)]),
            )
            nc.sync.dma_start(
                out=xr_all[:TAIL, NT * IN_DIM :],
                in_=xs[NT * 128 : NT * 128 + TAIL, :],
            )
            xb_all = cst.tile([128, (DPC // 128 + 1) * IN_DIM], bf16)
            nc.vector.tensor_copy(out=xb_all[:], in_=xr_all[:])
            nc.sync.dma_start(
                out=_v(xb_out[0:128, :], [(128 * IN_DIM, NT), (1, IN_DIM)]),
                in_=_v(xb_all[:], [(IN_DIM, NT), (1, IN_DIM)]),
            )
            nc.sync.dma_start(
                out=xb_out[NT * 128 : NT * 128 + TAIL, :],
                in_=xb_all[:TAIL, NT * IN_DIM :],
            )

            ab_all = cst.tile([128, len(ntile) * 2 * HEADS], bf16)
            n0 = 0
            for ti, nt in enumerate(ntile):
                a_ps = ps.tile([128, 2 * HEADS], f32, name="a_ps")
                nc.tensor.matmul(
                    out=a_ps[:nt, :],
                    lhsT=xTb_sb[:, n0 : n0 + nt],
                    rhs=vab_sb[:],
                    start=True,
                    stop=True,
                )
                nc.scalar.copy(
                    out=ab_all[:nt, ti * 2 * HEADS : (ti + 1) * 2 * HEADS],
                    in_=a_ps[:nt, :],
                )
                n0 += nt
            nc.sync.dma_start(
                out=_v(asad[0:128, :], [(128 * 2 * HEADS, NT), (1, 2 * HEADS)]),
                in_=_v(ab_all[:], [(2 * HEADS, NT), (1, 2 * HEADS)]),
            )
            nc.sync.dma_start(
                out=asad[NT * 128 : NT * 128 + TAIL, :],
                in_=ab_all[:TAIL, NT * 2 * HEADS : (NT + 1) * 2 * HEADS],
            )
    _split_waits(nc)
    return nc


def build_prog2(reps=1):
    """Per-core edge program v4: one 128-edge tile per 8-dst block, 16 blocks
    per epilogue group quad-stacked into 4 psum col-regions of 128 (d,h)
    partitions each. Host bin-packs dsts into blocks so tiles are ~full.
    """
    nc = bass.Bass()
    f32, bf16 = mybir.dt.float32, mybir.dt.bfloat16
    tcols = NBLK
    g_dram = nc.dram_tensor("g", [128, tcols * ROW], bf16, kind="ExternalInput")
    dm2_dram = nc.dram_tensor("dm2", [128, tcols * 2], bf16, kind="ExternalInput")
    iota2_dram = nc.dram_tensor("iota2", [128, 2 * BLK], bf16, kind="ExternalInput")
    w_dram = nc.dram_tensor("wnat", [IN_DIM, HH], f32, kind="ExternalInput")
    fw1_dram = nc.dram_tensor("fw1", [128, OUT_DIM], f32, kind="ExternalInput")
    fw2_dram = nc.dram_tensor("fw2", [128, OUT_DIM], f32, kind="ExternalInput")
    fb_dram = nc.dram_tensor("fbr", [128, OUT_DIM], f32, kind="ExternalInput")
    id_dram = nc.dram_tensor("id128", [128, 128], bf16, kind="ExternalInput")
    out_dram = nc.dram_tensor("out", [NG * 128, OUT_DIM], bf16, kind="ExternalOutput")

    with tile.TileContext(nc) as tc:
        with (
            tc.tile_pool(name="cst", bufs=1) as cst,
            tc.tile_pool(name="sb", bufs=10) as sb,
            tc.tile_pool(name="sb2", bufs=8) as sb2,
            tc.tile_pool(name="ps", bufs=2, space="PSUM") as ps,
            tc.tile_pool(name="ps2", bufs=3, space="PSUM") as ps2,
        ):
            dm2_sb = cst.tile([128, tcols * 2], bf16)
            nc.sync.dma_start(out=dm2_sb[:], in_=dm2_dram[:, :])
            iota2_sb = cst.tile([128, 2 * BLK], bf16)
            nc.sync.dma_start(out=iota2_sb[:], in_=iota2_dram[:, :])
            w_sb = cst.tile([IN_DIM, HH], f32)
            nc.sync.dma_start(out=w_sb[:], in_=w_dram[:, :])
            wb_sb = cst.tile([IN_DIM, HH], bf16)
            nc.scalar.copy(out=wb_sb[:], in_=w_sb[:])
            fw1_sb = cst.tile([128, OUT_DIM], f32)
            nc.sync.dma_start(out=fw1_sb[:], in_=fw1_dram[:, :])
            fw2_sb = cst.tile([128, OUT_DIM], f32)
            nc.sync.dma_start(out=fw2_sb[:], in_=fw2_dram[:, :])
            fw1b_sb = cst.tile([128, OUT_DIM], bf16)
            nc.scalar.copy(out=fw1b_sb[:], in_=fw1_sb[:])
            fw2b_sb = cst.tile([128, OUT_DIM], bf16)
            nc.scalar.copy(out=fw2b_sb[:], in_=fw2_sb[:])
            fb_sb = cst.tile([128, OUT_DIM], f32)
            nc.sync.dma_start(out=fb_sb[:], in_=fb_dram[:, :])
            id_sb = cst.tile([128, 128], bf16)
            nc.sync.dma_start(out=id_sb[:], in_=id_dram[:, :])
            ones1_sb = cst.tile([128, 128], bf16)
            nc.vector.memset(ones1_sb[:], 1.0)
            fbb_sb = cst.tile([128, OUT_DIM], bf16)
            nc.scalar.copy(out=fbb_sb[:], in_=fb_sb[:])

            def load_group(gi):
                    soff = gi * GBLK
                    g_sb = sb.tile([128, GBLK * ROW], bf16, name="g_sb", tag="g")
                    nc.sync.dma_start(
                        out=g_sb[:],
                        in_=g_dram[:, soff * ROW : (soff + GBLK) * ROW],
                    )
                    return g_sb

            def prep(gi, g_tile):
                    # alpha/one-hot prep for one 16-block group
                    soff = gi * GBLK
                    sT = GBLK
                    g_sb = g_tile[:, 0 : GBLK * ROW]
                    # alpha = exp(leakyrelu(as + ad)) : [e, (t,h)]
                    q_sb = sb.tile([128, sT * HEADS], bf16, name="q_sb", tag="q")
                    nc.vector.tensor_tensor(
                        out=q_sb[:],
                        in0=_v(g_sb, [(ROW, sT), (1, HEADS)], offset=AS_OFF),
                        in1=_v(g_sb, [(ROW, sT), (1, HEADS)], offset=AD_OFF),
                        op=mybir.AluOpType.add,
                    )
                    lr_sb = sb.tile([128, sT * HEADS], bf16, name="lr_sb", tag="lr")
                    nc.vector.scalar_tensor_tensor(
                        out=lr_sb[:], in0=q_sb[:], scalar=NEG_SLOPE, in1=q_sb[:],
                        op0=mybir.AluOpType.mult, op1=mybir.AluOpType.max,
                    )
                    al_sb = sb.tile([128, sT * HEADS], bf16, name="al_sb", tag="al")
                    nc.scalar.activation(
                        out=al_sb[:], in_=lr_sb[:],
                        func=mybir.ActivationFunctionType.Exp,
                    )
                    # S2[e, (t,d,j2)] = (dm[e,t] == d), j duplicated pair
                    s2_sb = sb.tile([128, sT * 2 * BLK], bf16, name="s2_sb", tag="s2")
                    nc.vector.tensor_tensor(
                        out=s2_sb[:],
                        in0=_v(iota2_sb[:], [(0, sT), (1, 2 * BLK)]),
                        in1=_v(dm2_sb[:], [(2, sT), (0, BLK), (1, 2)], offset=soff * 2),
                        op=mybir.AluOpType.is_equal,
                    )
                    # Salpha[e, (t, d, h4)] = S2 * alpha, two packed head-pair muls
                    sa_sb = sb.tile([128, sT * DHW], bf16, name="sa_sb", tag="sa")
                    for p2 in (0, 2):
                        nc.vector.tensor_tensor(
                            out=_v(sa_sb[:], [(DHW, sT), (HEADS, BLK), (1, 2)],
                                   offset=p2),
                            in0=_v(s2_sb[:], [(2 * BLK, sT), (2, BLK), (1, 2)]),
                            in1=_v(al_sb[:], [(HEADS, sT), (0, BLK), (1, 2)],
                                   offset=p2),
                            op=mybir.AluOpType.mult,
                        )
                    return g_sb, sa_sb

            def agg_group(ctx, b0, b1, agg_ps=None):
                    # block b (0..15) -> psum col-region b//4 (256-col pitch),
                    # partition quad (b%4)*32. col 128 of each region = den.
                    g_sb, sa_sb = ctx
                    if agg_ps is None:
                        agg_ps = ps.tile([128, 4 * 256], f32, name="agg_ps", tag="agg")
                    for b in range(b0, b1):
                        qo = (b % 4) * DHW
                        co = (b // 4) * 256
                        nc.tensor.matmul(
                            out=agg_ps[qo : qo + DHW, co : co + IN_DIM + 1],
                            lhsT=sa_sb[:, b * DHW : (b + 1) * DHW],
                            rhs=g_sb[:, b * ROW : b * ROW + IN_DIM + 1],
                            start=True,
                            stop=True,
                            tile_position=(0, qo),
                        )
                    return agg_ps

            def norm_stage(gi, agg_ps):
                    import os
                    if os.environ.get('SKIP_NORM'):
                        return None
                    # reciprocal over the 4 denominator cols + one fused
                    # normalize over all 4 regions (rc free-varying on DVE);
                    # frees agg_ps afterwards.
                    rc_sb = sb2.tile([128, 4], f32, name="rc", tag="rc")
                    nc.vector.reciprocal(
                        rc_sb[:],
                        _v(agg_ps[:], [(256, 4), (1, 1)], offset=IN_DIM),
                    )
                    nrm_sb = sb2.tile([128, 4 * 128], bf16, name="nrm", tag="nrm")
                    nc.vector.tensor_tensor(
                        out=nrm_sb[:],
                        in0=_v(agg_ps[:], [(256, 4), (1, IN_DIM)]),
                        in1=_v(rc_sb[:], [(1, 4), (0, IN_DIM)]),
                        op=mybir.AluOpType.mult,
                    )
                    return nrm_sb

            def tail_a(gi, nrm_sb):
                    if nrm_sb is None:
                        return None, None
                    # transposes + xpT evac. one psum bank per group: bytes
                    # 0:1024 first hold the 4 transposed blocks (bf16), then
                    # the per-head W outputs (fp32) overwrite them after the
                    # xpT evac; fc accumulates into bytes 1024:1536.
                    abo_ps = ps2.tile([128, 512], f32, name="abo_ps", tag="abo")
                    xp_ps = abo_ps[:].bitcast(bf16)   # [128, 1024] bf16 view
                    for r in range(4):
                        nc.tensor.transpose(
                            out=xp_ps[:, r * 128 : (r + 1) * 128],
                            in_=nrm_sb[:, r * 128 : (r + 1) * 128],
                            identity=id_sb[:],
                        )
                    xpT_sb = sb2.tile([128, 4 * 128], bf16, name="xpT", tag="xpT")
                    nc.scalar.copy(out=xpT_sb[:], in_=xp_ps[:, 0:512])
                    return abo_ps, xpT_sb

            def tail_b(gi, abo_ps, xpT_sb):
                    if abo_ps is None:
                        return None, None
                    # per-head W matmuls: xpT col = r*128 + q*32 + d*4 + h
                    for h in range(HEADS):
                        nc.tensor.matmul(
                            out=abo_ps[(h % 2) * HID : (h % 2 + 1) * HID,
                                       (h // 2) * 128 : (h // 2 + 1) * 128],
                            lhsT=wb_sb[:, h * HID : (h + 1) * HID],
                            rhs=_v(xpT_sb[:], [(128, 4), (DHW, 4), (HEADS, BLK)],
                                   offset=h),
                            start=True,
                            stop=True,
                        )
                    rl_sb = sb2.tile([128, 2 * 128], bf16, name="rl", tag="rl")
                    nc.scalar.activation(
                        out=rl_sb[:], in_=abo_ps[:, 0:256],
                        func=mybir.ActivationFunctionType.Relu,
                    )
                    return abo_ps, rl_sb

            ou_state = {}

            def tail_c(gi, abo_ps, rl_sb):
                    if abo_ps is None:
                        return
                    o_ps = abo_ps[:, 256:384]
                    nc.tensor.matmul(
                        out=o_ps, lhsT=rl_sb[:, 0:128], rhs=fw1b_sb[:],
                        start=True, stop=False,
                    )
                    nc.tensor.matmul(
                        out=o_ps, lhsT=rl_sb[:, 128:256], rhs=fw2b_sb[:],
                        start=False, stop=False,
                    )
                    # + fc_b via a 1-partition ones matmul (PSUM accumulate)
                    nc.tensor.matmul(
                        out=o_ps, lhsT=ones1_sb[0:1, :], rhs=fbb_sb[0:1, :],
                        start=False, stop=True,
                    )
                    # batch 4 groups per output store so g-load dispatches
                    # rarely queue behind a store on the SP queue
                    if "tile" not in ou_state:
                        ou_state["tile"] = sb2.tile(
                            [128, 4 * OUT_DIM], bf16, name="ou4", tag="ou4"
                        )
                        ou_state["g0"] = gi
                    ou4 = ou_state["tile"]
                    sl = gi - ou_state["g0"]
                    nc.scalar.copy(
                        out=ou4[:, sl * OUT_DIM : (sl + 1) * OUT_DIM],
                        in_=abo_ps[:, 256:384],
                    )
                    if sl == 3 or gi == NG - 1:
                        g0 = ou_state["g0"]
                        nc.scalar.dma_start(
                            out=_v(
                                out_dram[g0 * 128 : g0 * 128 + 128, :],
                                [(128 * OUT_DIM, sl + 1), (1, OUT_DIM)],
                            ),
                            in_=_v(ou4[:], [(OUT_DIM, sl + 1), (1, OUT_DIM)]),
                        )
                        ou_state.clear()

            # software pipeline, two epilogue stages: after emitting group g's
            # prep+agg, emit norm (DVE) for g-1 and the PE/ACT tail for g-2.
            # Keeps every engine's in-order stream free of head-of-line waits.
            rep_cm = tc.For_i(0, reps, 1) if reps > 1 else None
            if rep_cm is not None:
                rep_cm.__enter__()
            if True:
                PF = 5   # dma prefetch depth (groups)
                loaded = {gi: load_group(gi) for gi in range(min(PF, NG))}
                pend_norm = []   # (gi, agg_ps)
                pend_a = []      # (gi, nrm_sb)
                pend_b = []      # (gi, abo_ps, xpT_sb)
                pend_c = []      # (gi, abo_ps, rl_sb)
                def flush(n_keep_a=0, n_keep_b=0, n_keep_c=0):
                    while len(pend_a) > n_keep_a:
                        gp, nrm = pend_a.pop(0)
                        pend_b.append((gp,) + tail_a(gp, nrm))
                    while len(pend_b) > n_keep_b:
                        gp, abo, xpT = pend_b.pop(0)
                        pend_c.append((gp,) + tail_b(gp, abo, xpT))
                    while len(pend_c) > n_keep_c:
                        gp, abo, rl = pend_c.pop(0)
                        tail_c(gp, abo, rl)
                for gi in range(NG):
                    if gi + PF < NG:
                        loaded[gi + PF] = load_group(gi + PF)
                    ctx = prep(gi, loaded.pop(gi))
                    # interleave tail PE work with this group's agg matmuls
                    if pend_a:
                        gp, nrm = pend_a.pop(0)
                        pend_b.append((gp,) + tail_a(gp, nrm))
                    agg = agg_group(ctx, 0, 8)
                    if pend_b:
                        gp, abo, xpT = pend_b.pop(0)
                        pend_c.append((gp,) + tail_b(gp, abo, xpT))
                    agg_group(ctx, 8, 16, agg)
                    if pend_c:
                        gp, abo, rl = pend_c.pop(0)
                        tail_c(gp, abo, rl)
                    pend_norm.append((gi, agg))
                    if len(pend_norm) > 1:
                        gp, aggp = pend_norm.pop(0)
                        pend_a.append((gp, norm_stage(gp, aggp)))
                for gp, aggp in pend_norm:
                    pend_a.append((gp, norm_stage(gp, aggp)))
                flush()
            if rep_cm is not None:
                rep_cm.__exit__(None, None, None)
    _split_waits(nc)
    return nc


def _pack_core(dsts_sorted_desc, deg, nbins, cap=128, max_items=BLK):
    """Snake-stratified assignment + repair swaps so every bin has
    <= max_items dsts and <= cap edges. Returns list of dst-lists."""
    bins = [[] for _ in range(nbins)]
    b, direction = 0, 1
    for d in dsts_sorted_desc:
        bins[b].append(d)
        b += direction
        if b == nbins:
            b, direction = nbins - 1, -1
        elif b < 0:
            b, direction = 0, 1
    sums = np.array([deg[bb].sum() for bb in bins], np.int64)
    cnt = np.array([len(bb) for bb in bins], np.int64)
    for _ in range(100000):
        over = np.where(sums > cap)[0]
        if len(over) == 0:
            break
        i = over[np.argmax(sums[over])]
        excess = sums[i] - cap
        done = False
        for a in sorted(bins[i], key=lambda d: -deg[d]):
            under = np.where(sums <= cap - 1)[0]
            ju = under[np.argsort(sums[under])]
            for j in ju[:64]:
                for bidx, bd in enumerate(bins[j]):
                    da, db = deg[a], deg[bd]
                    if da - db >= excess and sums[j] - db + da <= cap:
                        bins[i].remove(a)
                        bins[j].pop(bidx)
                        bins[i].append(bd)
                        bins[j].append(a)
                        sums[i] += db - da
                        sums[j] += da - db
                        done = True
                        break
                if done:
                    break
            if done:
                break
        if not done:
            for a in sorted(bins[i], key=lambda d: -deg[d]):
                room = np.where((sums + deg[a] <= cap) & (cnt < max_items))[0]
                if len(room):
                    j = room[np.argmax(sums[room])]
                    bins[i].remove(a)
                    bins[j].append(a)
                    sums[i] -= deg[a]
                    sums[j] += deg[a]
                    cnt[i] -= 1
                    cnt[j] += 1
                    done = True
                    break
            if not done:
                raise RuntimeError("bin packing failed; raise NBLK")
    assert (sums <= cap).all() and (cnt <= max_items).all()
    return bins


def _host_prep(edge_index):
    """Index-only prep v4: self loops, degree-balanced dst->core snake,
    per-core bin-packing of dsts into NBLK 8-dst/128-edge tiles.
    Returns per-core (sid, dmod) slot arrays plus row_dst[NCORES, NG*128]
    (destination node per output slot, -1 for empty)."""
    src = np.concatenate(
        [np.asarray(edge_index[0], np.int64), np.arange(N, dtype=np.int64)]
    ).astype(np.int32)
    dst = np.concatenate(
        [np.asarray(edge_index[1], np.int64), np.arange(N, dtype=np.int64)]
    ).astype(np.int32)
    deg = np.bincount(dst, minlength=N)
    # per-dst edge lists via dst sort
    order = np.argsort(dst, kind="stable")
    src_s = src[order]
    dst_start = np.zeros(N + 1, np.int64)
    dst_start[1:] = np.cumsum(deg)

    dorder = np.argsort(-deg, kind="stable")
    snake = np.tile(
        np.concatenate([np.arange(NCORES), np.arange(NCORES - 1, -1, -1)]),
        N // (2 * NCORES) + 1,
    )[:N]
    core_of = np.empty(N, np.int32)
    core_of[dorder] = snake

    planes = []
    row_dst = np.full((NCORES, NG * 128), -1, np.int64)
    for k in range(NCORES):
        dsts_k = dorder[core_of[dorder] == k]  # desc by degree
        bins = _pack_core(dsts_k, deg, NBLK)
        sid = np.full(NBLK * 128, N, np.int32)
        did = np.full(NBLK * 128, N, np.int32)
        dmod = np.zeros(NBLK * 128, np.int32)
        for b, bl in enumerate(bins):
            gi, bg = b // GBLK, b % GBLK
            o = b * 128
            pos = 0
            for di, d in enumerate(bl):
                c = deg[d]
                sid[o + pos : o + pos + c] = src_s[dst_start[d] : dst_start[d] + c]
                did[o + pos : o + pos + c] = d
                dmod[o + pos : o + pos + c] = di
                pos += c
                row_dst[k, gi * 128 + (bg // 4) * 32 + (bg % 4) * 8 + di] = d
            # poison slots point at empty dmod slots (or 0 if bin is full)
            if pos < 128:
                dmod[o + pos : o + 128] = len(bl) % BLK
        planes.append((sid, did, dmod))
    return planes, row_dst


def prepare_in2(x, edge_index, W, att_src, att_dst, bias, fc_w, fc_b):
    """Run prog1 + host index shuffle; returns (in2, tcols, blk_off, blk_T)."""
    x = np.asarray(x, np.float32)
    W = np.asarray(W, np.float32)
    att_src = np.asarray(att_src, np.float32)
    att_dst = np.asarray(att_dst, np.float32)
    bias = np.asarray(bias, np.float32)
    fc_w = np.asarray(fc_w, np.float32)
    fc_b = np.asarray(fc_b, np.float32)

    xT = np.ascontiguousarray(x.T)                             # [128, N]
    attsr = np.tile(att_src.reshape(1, -1), (128, 1)).astype(np.float32)
    attdr = np.tile(att_dst.reshape(1, -1), (128, 1)).astype(np.float32)

    # ---- program 1: per-node [xb | as | ad] table shards
    nc1 = build_prog1()
    in1 = []
    for k in range(NCORES):
        in1.append(
            {
                "xTs": np.ascontiguousarray(xT[:, k * DPC : (k + 1) * DPC]),
                "xs": np.ascontiguousarray(x[k * DPC : (k + 1) * DPC, :]),
                "wnat": W,
                "attsr": attsr,
                "attdr": attdr,
            }
        )
    r1 = run_bass_kernel_spmd(nc1, in1, core_ids=list(range(NCORES)))
    xb = np.empty((N + 1, IN_DIM), np.uint16)
    asad = np.empty((N + 1, 2 * HEADS), np.uint16)
    for k in range(NCORES):
        xb[k * DPC : (k + 1) * DPC] = r1.results[k]["xb"].view(np.uint16)
        asad[k * DPC : (k + 1) * DPC] = r1.results[k]["asad"].view(np.uint16)
    # poison row: x=0, as=AS_PAD, ad=0
    xb[N, :] = 0
    asad[N, :HEADS] = np.array(AS_PAD, BF16).view(np.uint16)
    asad[N, HEADS:] = 0

    # ---- host: per-edge plane assembly (byte-level index shuffle only)
    planes, row_dst = _host_prep(edge_index)
    tcols = NBLK
    one_bf16 = np.array(1.0, BF16).view(np.uint16)
    iota2 = np.tile(
        np.repeat(np.arange(BLK, dtype=np.float32), 2).astype(BF16), (128, 1)
    )
    id128 = np.eye(128, dtype=np.float32).astype(BF16)
    fbr = np.tile(fc_b.reshape(1, -1), (128, 1)).astype(np.float32)
    # note: bias input is all-zero in this problem; fold would go into the
    # relu stage if nonzero.
    assert np.all(bias == 0.0), "nonzero GAT bias not implemented in v2"
    in2 = []
    for k in range(NCORES):
        sid, did, dmod = planes[k]
        rows = np.empty((tcols * 128, ROW), np.uint16)
        rows[:, :IN_DIM] = xb[sid]
        rows[:, ONE_OFF] = one_bf16
        rows[:, AS_OFF : AS_OFF + HEADS] = asad[sid, :HEADS]
        rows[:, AD_OFF : AD_OFF + HEADS] = asad[did, HEADS:]
        rows[:, ROW - 1] = 0
        g = np.ascontiguousarray(
            rows.reshape(tcols, 128, ROW).transpose(1, 0, 2).reshape(128, tcols * ROW)
        )
        dm2 = np.ascontiguousarray(
            np.repeat(dmod.astype(np.float32).astype(BF16), 2)
            .reshape(tcols, 128, 2).transpose(1, 0, 2).reshape(128, tcols * 2)
        )
        in2.append(
            {
                "g": g.view(BF16),
                "dm2": dm2,
                "iota2": iota2,
                "wnat": W,
                "fw1": np.ascontiguousarray(fc_w[:128, :]),
                "fw2": np.ascontiguousarray(fc_w[128:, :]),
                "fbr": fbr,
                "id128": id128,
            }
        )

    return in2, row_dst


def run_gat(x, edge_index, W, att_src, att_dst, bias, fc_w, fc_b, reps=1):
    in2, row_dst = prepare_in2(
        x, edge_index, W, att_src, att_dst, bias, fc_w, fc_b
    )
    nc2 = build_prog2(reps=reps)
    r2 = run_bass_kernel_spmd(nc2, in2, core_ids=list(range(NCORES)))
    out = np.empty((N, OUT_DIM), np.float32)
    for k in range(NCORES):
        rows = np.asarray(r2.results[k]["out"], dtype=np.float32)  # [NG*128, OUT]
        valid = row_dst[k] >= 0
        out[row_dst[k][valid]] = rows[valid]
    return out


def kernel(x, edge_index, W, att_src, att_dst, bias, fc_w, fc_b):
    return run_gat(x, edge_index, W, att_src, att_dst, bias, fc_w, fc_b, reps=1)

